# revision 7
# baseline (speedup 1.0000x reference)
"""Trainium2 Bass kernel for nn_ConditionalNFEncoder.

Computes, for inputs trend/seasonal/residual [B, T]:
  feat_trend    = trend[..., None] * Wt[:, 0] + bt        # [B, T, D]
  feat_seasonal = seasonal[..., None] * Ws[:, 0] + bs     # [B, T, D]
  lp            = MADE-flow log-prob of residual given shifted residual
  out           = concat([feat_trend, feat_seasonal, lp[..., None]], -1)

Key structural facts exploited here:

1. The flow transform is affine in x given the context c: each step applies
   z <- s_i(c) z + t_i(c), so  lp(x, c) = -(A(c)x + B(c))^2/2 - log(2pi)/2
   + L(c) = P2(c) x^2 + P1(c) x + P0(c), where P2/P1/P0 are smooth scalar
   functions of the scalar c.  With the problem's weight scale they are
   near-constant over the observed c range, so a degree-3 polynomial fit
   (computed on the host from the weights alone, validated on a dense grid
   at build time) replaces the whole per-token MLP: ~20 small DVE ops on
   token-major [128, 64] tiles cover all 8192 tokens of a core.

2. Each feature half-row is a scalar-affine image: trend_t*Wt+bt depends on
   the token only through one scalar.  Quantizing that scalar to 256 levels
   (err <= step/2 * max|W| ~ 2e-4, far under tolerance) turns feature
   generation into a 512-byte table-row gather.  Two supertiles' worth of
   features are produced by `dma_gather` straight from 256x512 fp8 tables
   in DRAM — zero compute-engine cost — trading spare HBM-read bandwidth
   for scarce ACT/DVE drain time.

3. The remaining six supertiles run on the PE: the K=3 contraction
   ([trend, seasonal, 1] x [Wt|0 / 0|Ws / bt|bs]) is packed 4-chunks-at-a-
   time into the 128x128 array with row tiling (tile_position=(32i, 0)), so
   a group of four 128-token chunks costs about one matmul span.  PSUM is
   drained by ACT and DVE (split by measured rate) directly to fp8-e4m3:
   |feat| <= 0.21 so fp8 abs err stays under 0.014 against the 2e-2 *
   max|out| ~= 0.042 tolerance, and fp8 halves the HBM write traffic.  The
   log-prob column is written bf16.  Output DMAs are HWDGE (sync engine)
   into DRAM mirrors of the SBUF tiles (fully contiguous); the host
   up-casts to fp32 and reassembles.

Sharding: pure data parallel over B across 8 NeuronCores (4 rows each).
"""

import numpy as np
import ml_dtypes

import concourse.bass as bass
import concourse.bacc as bacc
import concourse.tile as tile
from concourse import mybir
from concourse._compat import with_exitstack
from concourse.bass_utils import run_bass_kernel_spmd

# Problem constants (hardcoded per contract).
B, T, D, H, S, NBLK = 32, 2048, 512, 64, 3, 2
NCORES = 8
BP = B // NCORES            # batch rows per core = 4
N = BP * T                  # tokens per core = 8192
NCH = N // 128              # 128-token chunks per core = 64
NST = 8                     # supertiles per core (1024 tokens each)
NPE_ST = 6                  # supertiles computed on the PE
NG_ST = NST - NPE_ST        # supertiles gathered from the fp8 tables
TROWS = 256                 # table rows (quantization levels)
LOG_2PI = float(np.log(2.0 * np.pi))

f32 = mybir.dt.float32
bf16 = mybir.dt.bfloat16
f8 = mybir.dt.float8e4
i16 = mybir.dt.int16
AF = mybir.ActivationFunctionType
OP = mybir.AluOpType

# Per-chunk drain engine for the 48 PE chunks: ACT x28, DVE x20.
DRAIN = [True, True, True, False] * 8 + [True, False, False, False] * 4


def _flow_scale_shift(inp, c):
    """Exact per-step scale/shift of the flow as functions of context c [M]."""
    A = np.ones_like(c)
    Bv = np.zeros_like(c)
    L = np.zeros_like(c)
    cc = c[:, None]
    for i in range(S):
        h = cc @ inp["Wc0"][i].T.astype(np.float64) + (inp["bc0"][i] + inp["b_init"][i])
        for j in range(NBLK):
            t = np.maximum(h, 0) @ inp["W1"][i, j].T.astype(np.float64) + inp["b1"][i, j]
            t = np.maximum(t, 0) @ inp["W2"][i, j].T.astype(np.float64) + inp["b2"][i, j]
            g = cc @ inp["Wcb"][i, j].T.astype(np.float64) + inp["bcb"][i, j]
            h = h + t / (1.0 + np.exp(-g))
        out = np.maximum(h, 0) @ inp["Wf"][i].T.astype(np.float64) + inp["bf"][i]
        s = np.log1p(np.exp(out[:, 0])) + 1e-3
        A = s * A
        Bv = s * Bv + out[:, 1]
        L = L + np.log(s)
    return A, Bv, L


def _fit_lp_polys(inp, c_lo, c_hi):
    """Degree-3 fits of P2/P1/P0 over u = (c-mid)/half; coefficients in the
    power basis (Horner-ready), validated on a dense grid."""
    mid, half = (c_lo + c_hi) / 2.0, max((c_hi - c_lo) / 2.0, 1e-9)
    grid = np.linspace(c_lo, c_hi, 4096).astype(np.float64)
    A, Bv, L = _flow_scale_shift(inp, grid)
    P2 = -0.5 * A * A
    P1 = -A * Bv
    P0 = -0.5 * Bv * Bv + L - 0.5 * LOG_2PI
    u = (grid - mid) / half
    deg = 3
    while True:
        cfs = [np.polynomial.chebyshev.chebfit(u, P, deg) for P in (P2, P1, P0)]
        errs = [np.abs(np.polynomial.chebyshev.chebval(u, cf) - P).max()
                for cf, P in zip(cfs, (P2, P1, P0))]
        # conservative worst-case lp error over the c range for |x| <= 0.5
        if errs[0] * 0.25 + errs[1] * 0.5 + errs[2] < 2e-3 or deg >= 9:
            break
        deg += 2
    polys = [np.polynomial.chebyshev.cheb2poly(cf)[::-1] for cf in cfs]  # k_deg..k_0
    return polys, mid, half


@with_exitstack
def _body(ctx, tc, polys, mid, half, qt, yf, ygt, ygs, ylp,
          tso4, rh4, xc, xw, tbt, tbs):
    nc = tc.nc

    const = ctx.enter_context(tc.tile_pool(name="const", bufs=1))
    io = ctx.enter_context(tc.tile_pool(name="io", bufs=3))
    gg = ctx.enter_context(tc.tile_pool(name="gg", bufs=1))
    zp = ctx.enter_context(tc.tile_pool(name="zp", bufs=1))
    pq = ctx.enter_context(tc.tile_pool(name="pq", bufs=1, space="PSUM"))

    # ---- constants into SBUF ----
    tso4_sb = const.tile([128, NPE_ST * 2 * 128], bf16)
    nc.sync.dma_start(out=tso4_sb, in_=tso4)
    rh4_sb = const.tile([128, 2 * D], bf16)
    nc.sync.dma_start(out=rh4_sb, in_=rh4)
    xc_sb = const.tile([128, 2 * NCH], f32)
    nc.sync.dma_start(out=xc_sb, in_=xc)
    NW = NG_ST * 2 * 64          # idx columns (wrapped): 2 supertiles x 2 halves
    xw_sb = const.tile([128, NW], f32)
    nc.sync.dma_start(out=xw_sb, in_=xw)

    # ---- gather indices: idx = clamp(round((v - lo)/step), 0, TROWS-1) ----
    (tlo, tstep, slo, sstep) = qt
    idxf = zp.tile([128, NW], f32, tag="idxf")
    nc.vector.tensor_scalar(idxf[:, 0:NW // 2], xw_sb[:, 0:NW // 2],
                            1.0 / tstep, 0.5 - tlo / tstep, OP.mult, OP.add)
    nc.vector.tensor_scalar(idxf[:, NW // 2:NW], xw_sb[:, NW // 2:NW],
                            1.0 / sstep, 0.5 - slo / sstep, OP.mult, OP.add)
    idxc = zp.tile([128, NW], f32, tag="idxc")
    nc.vector.tensor_scalar(idxc, idxf, 0.0, float(TROWS) - 0.51, OP.max, OP.min)
    idxs = zp.tile([128, NW], i16, tag="idxs")
    nc.vector.tensor_copy(idxs, idxc)

    # ---- gathered supertiles: 512 B fp8 table rows straight from DRAM ----
    got = [None] * NG_ST
    gos = [None] * NG_ST
    for st in range(NG_ST):
        got[st] = gg.tile([128, 8 * D], f8, tag=f"got{st}", name=f"got{st}")
        nc.gpsimd.dma_gather(got[st].rearrange("p (j e) -> p j e", e=D), tbt,
                             idxs[:, st * 64:(st + 1) * 64], 1024, 1024, D,
                             elem_step=D)
        gos[st] = gg.tile([128, 8 * D], f8, tag=f"gos{st}", name=f"gos{st}")
        nc.gpsimd.dma_gather(gos[st].rearrange("p (j e) -> p j e", e=D), tbs,
                             idxs[:, (NG_ST + st) * 64:(NG_ST + st + 1) * 64],
                             1024, 1024, D, elem_step=D)

    # ---- lp chain: all-DVE, token-major [128, 64] ----
    x_v = xc_sb[:, 0:NCH]
    c_v = xc_sb[:, NCH:2 * NCH]
    zsh = [128, NCH]

    def cubic(name, ks):
        t = zp.tile(zsh, f32, tag=f"{name}a")
        nc.vector.tensor_scalar(t, u_t, float(ks[0]), float(ks[1]), OP.mult, OP.add)
        steps = [t]
        for d in range(2, len(ks)):
            m = zp.tile(zsh, f32, tag=f"{name}m{d}")
            nc.vector.tensor_tensor(m, steps[-1], u_t, OP.mult)
            a = zp.tile(zsh, f32, tag=f"{name}s{d}")
            nc.vector.tensor_scalar_add(a, m, float(ks[d]))
            steps.append(a)
        return steps[-1]

    u_t = zp.tile(zsh, f32, tag="u")
    nc.vector.tensor_scalar(u_t, c_v, 1.0 / half, -mid / half, OP.mult, OP.add)
    p2_t = cubic("p2", polys[0])
    p1_t = cubic("p1", polys[1])
    p0_t = cubic("p0", polys[2])
    m1 = zp.tile(zsh, f32, tag="m1")
    nc.vector.tensor_tensor(m1, p2_t, x_v, OP.mult)
    s1 = zp.tile(zsh, f32, tag="s1")
    nc.vector.tensor_tensor(s1, m1, p1_t, OP.add)
    m2 = zp.tile(zsh, f32, tag="m2")
    nc.vector.tensor_tensor(m2, s1, x_v, OP.mult)
    lp_bf = zp.tile(zsh, bf16, tag="lpbf")
    nc.vector.tensor_tensor(lp_bf, m2, p0_t, OP.add)

    # ---- PE supertiles + interleaved output DMAs ----
    W2 = 2 * D
    for s in range(NPE_ST):
        outt = io.tile([128, 8 * W2], f8, tag="outt")
        for gq in range(2):              # two groups of 4 chunks per supertile
            q = 2 * s + gq
            ps = [pq.tile([128, W2], f32, tag=f"ps{i}", name=f"ps{i}")
                  for i in range(4)]
            for h in range(2):
                for i in range(4):
                    nc.tensor.matmul(
                        ps[i][:, h * D:(h + 1) * D],
                        tso4_sb[32 * i:32 * i + 3, q * 128:(q + 1) * 128],
                        rh4_sb[32 * i:32 * i + 3, h * D:(h + 1) * D],
                        start=True, stop=True, tile_position=(32 * i, 0))
            for i in range(4):
                k = 4 * gq + i
                dst = outt[:, k * W2:(k + 1) * W2]
                if DRAIN[4 * q + i]:
                    nc.scalar.copy(dst, ps[i])
                else:
                    nc.vector.tensor_copy(dst, ps[i])
        nc.sync.dma_start(out=yf[s], in_=outt)
        if s == 0:
            nc.sync.dma_start(out=ylp, in_=lp_bf)
        elif s == 1:
            nc.sync.dma_start(out=ygt[0], in_=got[0])
            nc.sync.dma_start(out=ygs[0], in_=gos[0])
        elif s == 3:
            nc.sync.dma_start(out=ygt[1], in_=got[1])
            nc.sync.dma_start(out=ygs[1], in_=gos[1])


def _build_module(polys, mid, half, qt):
    nc = bacc.Bacc("TRN2", target_bir_lowering=False, debug=False,
                   enable_asserts=False, num_devices=NCORES)
    W2 = 2 * D
    yf = nc.dram_tensor("yf", [NPE_ST, 128, 8 * W2], f8, kind="ExternalOutput").ap()
    ygt = nc.dram_tensor("ygt", [NG_ST, 128, 8 * D], f8, kind="ExternalOutput").ap()
    ygs = nc.dram_tensor("ygs", [NG_ST, 128, 8 * D], f8, kind="ExternalOutput").ap()
    ylp = nc.dram_tensor("ylp", [128, NCH], bf16, kind="ExternalOutput").ap()
    tso4 = nc.dram_tensor("tso4", [128, NPE_ST * 2 * 128], bf16,
                          kind="ExternalInput").ap()
    rh4 = nc.dram_tensor("rh4", [128, W2], bf16, kind="ExternalInput").ap()
    xc = nc.dram_tensor("xc", [128, 2 * NCH], f32, kind="ExternalInput").ap()
    xw = nc.dram_tensor("xw", [128, NG_ST * 2 * 64], f32, kind="ExternalInput").ap()
    tbt = nc.dram_tensor("tbt", [TROWS, D], f8, kind="ExternalInput").ap()
    tbs = nc.dram_tensor("tbs", [TROWS, D], f8, kind="ExternalInput").ap()
    with tile.TileContext(nc) as tc:
        _body(tc, polys, mid, half, qt, yf, ygt, ygs, ylp, tso4, rh4, xc, xw,
              tbt, tbs)
    nc.compile()
    return nc


def _run(inputs, trace=False):
    trend = np.asarray(inputs["trend"], np.float32)
    seasonal = np.asarray(inputs["seasonal"], np.float32)
    residual = np.asarray(inputs["residual"], np.float32)
    prev = np.concatenate([np.zeros_like(residual[:, :1]), residual[:, :-1]], axis=1)

    polys, mid, half = _fit_lp_polys(
        inputs, float(prev.min()) - 1e-6, float(prev.max()) + 1e-6)

    # fp8 feature-row tables over 256 quantization levels of trend/seasonal
    def qparams(v):
        lo, hi = float(v.min()), float(v.max())
        return lo, max((hi - lo) / (TROWS - 1), 1e-9)
    tlo, tstep = qparams(trend)
    slo, sstep = qparams(seasonal)
    lv_t = tlo + tstep * np.arange(TROWS, dtype=np.float64)
    lv_s = slo + sstep * np.arange(TROWS, dtype=np.float64)
    tbt = (lv_t[:, None] * inputs["Wt"][:, 0] + inputs["bt"]).astype(
        ml_dtypes.float8_e4m3)
    tbs = (lv_s[:, None] * inputs["Ws"][:, 0] + inputs["bs"]).astype(
        ml_dtypes.float8_e4m3)
    qt = (tlo, tstep, slo, sstep)
    nc = _build_module(polys, mid, half, qt)

    rh = np.zeros((3, 2 * D), np.float32)
    rh[0, :D] = inputs["Wt"][:, 0]
    rh[1, D:] = inputs["Ws"][:, 0]
    rh[2, :D] = inputs["bt"]
    rh[2, D:] = inputs["bs"]
    rh4 = np.zeros((4, 32, 2 * D), np.float32)
    rh4[:, 0:3, :] = rh
    rh4 = rh4.reshape(128, 2 * D).astype(ml_dtypes.bfloat16)

    in_maps = []
    for cidx in range(NCORES):
        sl = slice(cidx * BP, (cidx + 1) * BP)
        tr_f = trend[sl].reshape(-1)
        se_f = seasonal[sl].reshape(-1)
        # stationary for the 6 PE supertiles (= first 12 groups of 4 chunks)
        tr = tr_f[:NPE_ST * 1024].reshape(NPE_ST * 2, 4, 128)
        se = se_f[:NPE_ST * 1024].reshape(NPE_ST * 2, 4, 128)
        tso4 = np.zeros((4, 32, NPE_ST * 2, 128), np.float32)
        tso4[:, 0] = tr.transpose(1, 0, 2)
        tso4[:, 1] = se.transpose(1, 0, 2)
        tso4[:, 2] = 1.0
        tso4 = tso4.reshape(128, NPE_ST * 2 * 128).astype(ml_dtypes.bfloat16)
        xc = np.empty((128, 2 * NCH), np.float32)
        xc[:, :NCH] = residual[sl].reshape(NCH, 128).T
        xc[:, NCH:] = prev[sl].reshape(NCH, 128).T
        # wrapped scalars for the gathered supertiles: value of token
        # (st*1024 + 16*jj + ch) at [16m+ch, st*64 + jj], all cores m alike
        xw = np.empty((128, NG_ST * 2 * 64), np.float32)
        for st in range(NG_ST):
            blk_t = tr_f[(NPE_ST + st) * 1024:(NPE_ST + st + 1) * 1024]
            blk_s = se_f[(NPE_ST + st) * 1024:(NPE_ST + st + 1) * 1024]
            wt = blk_t.reshape(64, 16).T            # [ch, jj]
            ws = blk_s.reshape(64, 16).T
            xw[:, st * 64:(st + 1) * 64] = np.tile(wt, (8, 1))
            xw[:, (NG_ST + st) * 64:(NG_ST + st + 1) * 64] = np.tile(ws, (8, 1))
        in_maps.append({"tso4": tso4, "rh4": rh4,
                        "xc": np.ascontiguousarray(xc),
                        "xw": np.ascontiguousarray(xw),
                        "tbt": tbt, "tbs": tbs})

    res = run_bass_kernel_spmd(nc, in_maps, core_ids=list(range(NCORES)),
                               trace=trace)
    W2 = 2 * D
    parts = []
    for r in res.results:
        fpe = np.asarray(r["yf"]).astype(np.float32)
        fpe = fpe.reshape(NPE_ST, 128, 8, W2).transpose(0, 2, 1, 3)
        gt = np.asarray(r["ygt"]).astype(np.float32).reshape(NG_ST, 128, 8, D)
        gs = np.asarray(r["ygs"]).astype(np.float32).reshape(NG_ST, 128, 8, D)
        fg = np.concatenate([gt, gs], axis=3).transpose(0, 2, 1, 3)
        feat = np.concatenate([fpe, fg], axis=0).reshape(N, W2)
        lp = np.asarray(r["ylp"]).astype(np.float32).T.reshape(N, 1)
        parts.append(np.concatenate([feat, lp], axis=1).reshape(BP, T, W2 + 1))
    return np.concatenate(parts, axis=0), res


def kernel(**inputs):
    out, _ = _run(inputs, trace=False)
    return out


# revision 15
# speedup vs baseline: 1.3125x; 1.3125x over previous
"""Trainium2 Bass kernel for nn_ConditionalNFEncoder.

Computes, for inputs trend/seasonal/residual [B, T]:
  feat_trend    = trend[..., None] * Wt[:, 0] + bt        # [B, T, D]
  feat_seasonal = seasonal[..., None] * Ws[:, 0] + bs     # [B, T, D]
  lp            = MADE-flow log-prob of residual given shifted residual
  out           = concat([feat_trend, feat_seasonal, lp[..., None]], -1)

Key structural facts exploited here:

1. The flow transform is affine in x given the context c: each step applies
   z <- s_i(c) z + t_i(c), so  lp(x, c) = -(A(c)x + B(c))^2/2 - log(2pi)/2
   + L(c) = P2(c) x^2 + P1(c) x + P0(c), where P2/P1/P0 are smooth scalar
   functions of the scalar c.  With the problem's weight scale they are
   near-constant over the observed c range, so a degree-3 polynomial fit
   (computed on the host from the weights alone, validated on a dense grid
   at build time) replaces the whole per-token MLP: ~20 small DVE ops on
   token-major [128, 64] tiles cover all 8192 tokens of a core.

2. The feature columns are a K=3 contraction ([trend, seasonal, 1] x
   [Wt|0 / 0|Ws / bt|bs]).  All 64 token-chunks per core run on the PE,
   packed 4-at-a-time into the 128x128 array with row tiling
   (tile_position=(32i, 0)): the four K=3 matmuls occupy disjoint 32-row
   bands and execute concurrently, so a group of four 128-token chunks
   costs about one matmul's span.  Host-side marshaling places each
   chunk's [trend/seasonal/ones] rows at partition offset 32i and
   replicates the moving operand across the four bands.

3. The kernel is then bound by the PSUM->SBUF drain pass and the output
   DMA.  Drains are split ACT/DVE (both ~1 col/cycle out of PSUM,
   dtype-insensitive) and write fp8-e4m3 directly: |feat| <= 0.21 so the
   fp8 abs err stays under 0.014 against the 2e-2 * max|out| ~= 0.042
   tolerance, and fp8 halves the HBM write traffic.  The log-prob column
   is written bf16.  Output DMAs are HWDGE (sync engine) into DRAM
   mirrors of the SBUF tiles (fully contiguous); the host up-casts and
   reassembles.

Sharding: pure data parallel over B across 8 NeuronCores (4 rows each).
"""

import numpy as np
import ml_dtypes

import concourse.bass as bass
import concourse.bacc as bacc
import concourse.tile as tile
from concourse import mybir
from concourse._compat import with_exitstack
from concourse.bass_utils import run_bass_kernel_spmd

# Problem constants (hardcoded per contract).
B, T, D, H, S, NBLK = 32, 2048, 512, 64, 3, 2
NCORES = 8
BP = B // NCORES            # batch rows per core = 4
N = BP * T                  # tokens per core = 8192
NCH = N // 128              # 128-token chunks per core = 64
NST = 8                     # supertiles per core (1024 tokens each)
NGRP = NCH // 4             # PE row-tile groups of 4 chunks = 16
LOG_2PI = float(np.log(2.0 * np.pi))

f32 = mybir.dt.float32
bf16 = mybir.dt.bfloat16
f8 = mybir.dt.float8e4
AF = mybir.ActivationFunctionType
OP = mybir.AluOpType

# Per-pair drain engine for the 32 [128, 2048] PSUM pairs: ACT x18, DVE x14.
DRAINP = [True, False] * 16
DRAINP[11] = DRAINP[23] = True


def _flow_scale_shift(inp, c):
    """Exact per-step scale/shift of the flow as functions of context c [M]."""
    A = np.ones_like(c)
    Bv = np.zeros_like(c)
    L = np.zeros_like(c)
    cc = c[:, None]
    for i in range(S):
        h = cc @ inp["Wc0"][i].T.astype(np.float64) + (inp["bc0"][i] + inp["b_init"][i])
        for j in range(NBLK):
            t = np.maximum(h, 0) @ inp["W1"][i, j].T.astype(np.float64) + inp["b1"][i, j]
            t = np.maximum(t, 0) @ inp["W2"][i, j].T.astype(np.float64) + inp["b2"][i, j]
            g = cc @ inp["Wcb"][i, j].T.astype(np.float64) + inp["bcb"][i, j]
            h = h + t / (1.0 + np.exp(-g))
        out = np.maximum(h, 0) @ inp["Wf"][i].T.astype(np.float64) + inp["bf"][i]
        s = np.log1p(np.exp(out[:, 0])) + 1e-3
        A = s * A
        Bv = s * Bv + out[:, 1]
        L = L + np.log(s)
    return A, Bv, L


def _fit_lp_polys(inp, c_lo, c_hi):
    """Degree-3 fits of P2/P1/P0 over u = (c-mid)/half; coefficients in the
    power basis (Horner-ready), validated on a dense grid."""
    mid, half = (c_lo + c_hi) / 2.0, max((c_hi - c_lo) / 2.0, 1e-9)
    grid = np.linspace(c_lo, c_hi, 4096).astype(np.float64)
    A, Bv, L = _flow_scale_shift(inp, grid)
    P2 = -0.5 * A * A
    P1 = -A * Bv
    P0 = -0.5 * Bv * Bv + L - 0.5 * LOG_2PI
    u = (grid - mid) / half
    deg = 3
    while True:
        cfs = [np.polynomial.chebyshev.chebfit(u, P, deg) for P in (P2, P1, P0)]
        errs = [np.abs(np.polynomial.chebyshev.chebval(u, cf) - P).max()
                for cf, P in zip(cfs, (P2, P1, P0))]
        # conservative worst-case lp error over the c range for |x| <= 0.5
        if errs[0] * 0.25 + errs[1] * 0.5 + errs[2] < 2e-3 or deg >= 9:
            break
        deg += 2
    polys = [np.polynomial.chebyshev.cheb2poly(cf)[::-1] for cf in cfs]  # k_deg..k_0
    return polys, mid, half


@with_exitstack
def _body(ctx, tc, polys, mid, half, yf, ylp, tso4, rh4, xc):
    nc = tc.nc

    const = ctx.enter_context(tc.tile_pool(name="const", bufs=1))
    io = ctx.enter_context(tc.tile_pool(name="io", bufs=3))
    zp = ctx.enter_context(tc.tile_pool(name="zp", bufs=1))
    pq = ctx.enter_context(tc.tile_pool(name="pq", bufs=1, space="PSUM"))

    # ---- constants into SBUF (rh4 first: smallest PE dependency) ----
    rh4_sb = const.tile([128, 2 * D], bf16)
    nc.sync.dma_start(out=rh4_sb, in_=rh4)
    tso4_sb = const.tile([128, NGRP * 128], bf16)
    nc.sync.dma_start(out=tso4_sb, in_=tso4)
    xc_sb = const.tile([128, 2 * NCH], f32)
    nc.sync.dma_start(out=xc_sb, in_=xc)

    x_v = xc_sb[:, 0:NCH]
    c_v = xc_sb[:, NCH:2 * NCH]

    # ---- lp chain: all-DVE, token-major [128, 64] ----
    zsh = [128, NCH]

    def cubic(name, ks):
        t = zp.tile(zsh, f32, tag=f"{name}a")
        nc.vector.tensor_scalar(t, u_t, float(ks[0]), float(ks[1]), OP.mult, OP.add)
        steps = [t]
        for d in range(2, len(ks)):
            m = zp.tile(zsh, f32, tag=f"{name}m{d}")
            nc.vector.tensor_tensor(m, steps[-1], u_t, OP.mult)
            a = zp.tile(zsh, f32, tag=f"{name}s{d}")
            nc.vector.tensor_scalar_add(a, m, float(ks[d]))
            steps.append(a)
        return steps[-1]

    u_t = zp.tile(zsh, f32, tag="u")
    nc.vector.tensor_scalar(u_t, c_v, 1.0 / half, -mid / half, OP.mult, OP.add)
    p2_t = cubic("p2", polys[0])
    p1_t = cubic("p1", polys[1])
    p0_t = cubic("p0", polys[2])
    m1 = zp.tile(zsh, f32, tag="m1")
    nc.vector.tensor_tensor(m1, p2_t, x_v, OP.mult)
    s1 = zp.tile(zsh, f32, tag="s1")
    nc.vector.tensor_tensor(s1, m1, p1_t, OP.add)
    m2 = zp.tile(zsh, f32, tag="m2")
    nc.vector.tensor_tensor(m2, s1, x_v, OP.mult)
    lp_bf = zp.tile(zsh, bf16, tag="lpbf")
    nc.vector.tensor_tensor(lp_bf, m2, p0_t, OP.add)

    # ---- features: 16 groups of 4 chunks, 4-way row-tiled on the PE ----
    # PSUM: two [128, 2048] pair tiles (4 banks each); drains at pair
    # granularity; one output tile + DMA per group (0.5 MB).
    W2 = 2 * D
    for q in range(NGRP):
        outt = io.tile([128, 4 * W2], f8, tag="outt")
        ps = [pq.tile([128, 2 * W2], f32, tag=f"ps{j}", name=f"ps{j}")
              for j in range(2)]
        for h in range(2):
            for i in range(4):
                nc.tensor.matmul(
                    ps[i // 2][:, (i % 2) * W2 + h * D:(i % 2) * W2 + (h + 1) * D],
                    tso4_sb[32 * i:32 * i + 3, q * 128:(q + 1) * 128],
                    rh4_sb[32 * i:32 * i + 3, h * D:(h + 1) * D],
                    start=True, stop=True, tile_position=(32 * i, 0))
        for j in range(2):
            dst = outt[:, (2 * j) * W2:(2 * j + 2) * W2]
            if DRAINP[2 * q + j]:
                nc.scalar.copy(dst, ps[j])
            else:
                nc.vector.tensor_copy(dst, ps[j])
        nc.sync.dma_start(out=yf[q], in_=outt)
        if q == 0:
            nc.sync.dma_start(out=ylp, in_=lp_bf)


def _build_module(polys, mid, half):
    nc = bacc.Bacc("TRN2", target_bir_lowering=False, debug=False,
                   enable_asserts=False, num_devices=NCORES)
    W2 = 2 * D
    yf = nc.dram_tensor("yf", [NGRP, 128, 4 * W2], f8, kind="ExternalOutput").ap()
    ylp = nc.dram_tensor("ylp", [128, NCH], bf16, kind="ExternalOutput").ap()
    tso4 = nc.dram_tensor("tso4", [128, NGRP * 128], bf16, kind="ExternalInput").ap()
    rh4 = nc.dram_tensor("rh4", [128, W2], bf16, kind="ExternalInput").ap()
    xc = nc.dram_tensor("xc", [128, 2 * NCH], f32, kind="ExternalInput").ap()
    with tile.TileContext(nc) as tc:
        _body(tc, polys, mid, half, yf, ylp, tso4, rh4, xc)
    nc.compile()
    return nc


def _run(inputs, trace=False):
    trend = np.asarray(inputs["trend"], np.float32)
    seasonal = np.asarray(inputs["seasonal"], np.float32)
    residual = np.asarray(inputs["residual"], np.float32)
    prev = np.concatenate([np.zeros_like(residual[:, :1]), residual[:, :-1]], axis=1)

    polys, mid, half = _fit_lp_polys(
        inputs, float(prev.min()) - 1e-6, float(prev.max()) + 1e-6)
    nc = _build_module(polys, mid, half)

    # moving operand, replicated into the four 32-partition bands
    rh = np.zeros((3, 2 * D), np.float32)
    rh[0, :D] = inputs["Wt"][:, 0]
    rh[1, D:] = inputs["Ws"][:, 0]
    rh[2, :D] = inputs["bt"]
    rh[2, D:] = inputs["bs"]
    rh4 = np.zeros((4, 32, 2 * D), np.float32)
    rh4[:, 0:3, :] = rh
    rh4 = rh4.reshape(128, 2 * D).astype(ml_dtypes.bfloat16)

    in_maps = []
    for cidx in range(NCORES):
        sl = slice(cidx * BP, (cidx + 1) * BP)
        # stationary: chunk (4q+i)'s [trend/seasonal/ones] rows at partitions 32i+j
        tr = trend[sl].reshape(NGRP, 4, 128)
        se = seasonal[sl].reshape(NGRP, 4, 128)
        tso4 = np.zeros((4, 32, NGRP, 128), np.float32)
        tso4[:, 0] = tr.transpose(1, 0, 2)
        tso4[:, 1] = se.transpose(1, 0, 2)
        tso4[:, 2] = 1.0
        tso4 = tso4.reshape(128, NGRP * 128).astype(ml_dtypes.bfloat16)
        xc = np.empty((128, 2 * NCH), np.float32)
        xc[:, :NCH] = residual[sl].reshape(NCH, 128).T
        xc[:, NCH:] = prev[sl].reshape(NCH, 128).T
        in_maps.append({"tso4": tso4, "rh4": rh4,
                        "xc": np.ascontiguousarray(xc)})

    res = run_bass_kernel_spmd(nc, in_maps, core_ids=list(range(NCORES)),
                               trace=trace)
    W2 = 2 * D
    parts = []
    for r in res.results:
        feat = np.asarray(r["yf"]).astype(np.float32)
        feat = feat.reshape(NGRP, 128, 4, W2).transpose(0, 2, 1, 3).reshape(N, W2)
        lp = np.asarray(r["ylp"]).astype(np.float32).T.reshape(N, 1)
        parts.append(np.concatenate([feat, lp], axis=1).reshape(BP, T, W2 + 1))
    return np.concatenate(parts, axis=0), res


def kernel(**inputs):
    out, _ = _run(inputs, trace=False)
    return out
